# revision 32
# baseline (speedup 1.0000x reference)
"""DINOv3 ViT attention (det tokens + hidden, partial RoPE) on 8 Trainium2 cores.

Strategy: data-parallel over batch (B=8 -> 1 batch element per core).
Per core, everything is bf16 on the matmul paths with fp32 accumulation:
  phase 1: q^T,k^T (feature-major, weights stationary) + RoPE fused from PSUM;
           v token-major (x^T stationary) with per-head 65-column layout
           (64 dims + a ones column used to compute softmax denominators).
  phase 2: per head: scores^T = k^T.T @ q^T (row-packed 2 heads in PE),
           exp on ScalarE (one wide ACTIVATE covering both heads via a
           strided dst AP; scale=1/8 folded),
           out_h = exp_sT.T @ [v_h | 1] accumulated over k tiles, then
           normalize by the ones-column sum (reciprocal + tensor_scalar).
  phase 3 (fused into the same context): PE-transpose attn_out to
           feature-major, o_proj (mapping B) + bias, DMA out.

RoPE trick: rotate_half needs a cross-partition swap of +-32 which DVE's
stream_shuffle cannot do (it permutes lanes within 32-partition blocks only).
We permute the head-dim order of q/k on the host (rows of q_w/dq_w/k_w and the
cos/sin tables) as [0:16, 32:48, 16:32, 48:64] per head, which puts every
rotate pair +-16 apart inside one 32-lane block. Scores are invariant to a
consistent d-permutation, so nothing downstream changes.

Weights are repacked on the host so every per-pair slice is a contiguous
DRAM region (2KB/partition descriptors instead of 256B strided reads).
"""
import numpy as np
import ml_dtypes
from contextlib import ExitStack

import concourse.mybir as mybir
import concourse.tile as tile
from concourse import bacc
from concourse.bass_utils import run_bass_kernel_spmd
from concourse.masks import make_identity

BF16 = mybir.dt.bfloat16
F32 = mybir.dt.float32
AF = mybir.ActivationFunctionType
OP = mybir.AluOpType

B = 8
NDET = 100
NHID = 1029
S = NDET + NHID            # 1129
SP = S + 1                 # 1130, padded (pad col never read)
E = 1024
H = 16
HD = 64
P = 1024
R0 = S - P                 # 105: first roped token
KT = E // 128              # 8
ST = (S + 127) // 128      # 9
SCALE = HD ** -0.5

Q_CHUNKS = [(0, 100, "dq"), (100, 512, "q"), (612, 512, "q"), (1124, 6, "q")]
K_CHUNKS = [(0, 512, "k"), (512, 512, "k"), (1024, 106, "k")]
SC_CHUNKS = [(0, 512), (512, 512), (1024, 105)]
E_CHUNKS = [(0, 512), (512, 512)]

# rotate_half pairs land +-16 apart within 32-lane blocks (see module docstring)
PERM64 = np.concatenate([np.arange(0, 16), np.arange(32, 48),
                         np.arange(16, 32), np.arange(48, 64)])
PERM_E = np.concatenate([h * 64 + PERM64 for h in range(H)])
SWAP_MASK = [(l + 16) % 32 for l in range(32)]

_CACHE = {}


def _sw(t, n=ST, full=128, last=S - 8 * 128):
    return full if t < n - 1 else last


def _build():
    nc = bacc.Bacc("TRN2", target_bir_lowering=False, debug=False, num_devices=B)
    dp = nc.declare_dram_parameter
    xT_d = dp("xT", [E, SP], BF16, False)
    # qkv weights pre-sliced per output E-tile: [m, p, k, e] contiguous
    wq_d = dp("wq", [KT, 128, KT, 128], BF16, False)
    wdq_d = dp("wdq", [KT, 128, KT, 128], BF16, False)
    wk_d = dp("wk", [KT, 128, KT, 128], BF16, False)
    # v/o weights pre-transposed to [p, k, e] contiguous
    wv_d = dp("wv", [128, KT, E], BF16, False)
    wo_d = dp("wo", [128, KT, E], BF16, False)
    cos2_d = dp("cos2", [128, P], BF16, False)
    sinS_d = dp("sinS", [128, P], BF16, False)
    qb2_d = dp("qb2", [128, KT], F32, False)
    qb2s_d = dp("qb2s", [128, KT], F32, False)
    dqb2_d = dp("dqb2", [128, KT], F32, False)
    vb_d = dp("vb", [1, E], BF16, False)
    ob_d = dp("ob", [1, E], BF16, False)
    out_d = dp("out", [S, E], F32, True)

    with tile.TileContext(nc) as tc, ExitStack() as octx:
        const = octx.enter_context(tc.tile_pool(name="const", bufs=1))
        qkv = octx.enter_context(tc.tile_pool(name="qkv", bufs=1))

        cos2_sb = const.tile([128, P], BF16, tag="cos2")
        sinS_sb = const.tile([128, P], BF16, tag="sinS")
        qb2_sb = const.tile([128, KT], F32, tag="qb2")
        qb2s_sb = const.tile([128, KT], F32, tag="qb2s")
        dqb2_sb = const.tile([128, KT], F32, tag="dqb2")
        vb_sb = const.tile([1, E], BF16, tag="vb")
        ob_sb = const.tile([1, E], BF16, tag="ob")
        ones_sb = const.tile([1, 128], BF16, tag="ones")
        nc.gpsimd.memset(ones_sb[:], 1.0)

        v_sb = qkv.tile([128, ST, H * 65], BF16, tag="v")
        # padded to 9*128 cols: the last 112-row XBAR transpose spills 7 cols
        aT_sb = [qkv.tile([128, ST * 128], BF16, tag=f"aT{et}", name=f"aT_{et}")
                 for et in range(KT)]  # feature-major attn out, per E-tile

        # ones column of v_aug (col 64 of each per-head 65-block)
        v4 = v_sb[:, :, :].rearrange("p s (h d) -> p s h d", d=65)
        nc.gpsimd.memset(v4[:, :, :, 64:65], 1.0)

        wapc = {"q": wq_d, "dq": wdq_d, "k": wk_d}

        # ------- phases 1+2+3 in one context -------
        with ExitStack() as ctx:
            wsl = ctx.enter_context(tc.tile_pool(name="wsl", bufs=2))
            xp = ctx.enter_context(tc.tile_pool(name="xp", bufs=1))
            qkp = ctx.enter_context(tc.tile_pool(name="qkp", bufs=3))
            psA = ctx.enter_context(tc.tile_pool(name="psA", bufs=1, space="PSUM"))
            rtmp = ctx.enter_context(tc.tile_pool(name="rtmp", bufs=1))
            attnp = ctx.enter_context(tc.tile_pool(name="attn", bufs=2))
            aop = ctx.enter_context(tc.tile_pool(name="aop", bufs=2))
            nrm = ctx.enter_context(tc.tile_pool(name="nrm", bufs=6))
            wvo = ctx.enter_context(tc.tile_pool(name="wvo", bufs=1))
            outp = ctx.enter_context(tc.tile_pool(name="outp", bufs=2))

            def load_w(pair, split=False):
                """DMA the three qkv weight slices for one E-tile."""
                wsb_m = {}
                for i, which in enumerate(("q", "dq", "k")):
                    wsb_m[which] = wsl.tile([128, KT, 128], BF16, tag="w" + which,
                                            name=f"w_{which}_{pair}")
                    eng = nc.gpsimd if (split and i > 0) else nc.sync
                    eng.dma_start(wsb_m[which][:], wapc[which].ap()[pair])
                return wsb_m

            # pair-0 weights lead both queues so the PE can start ASAP;
            # x tiles alternate across the two queues right behind them
            w0 = load_w(0, split=True)
            xT_ap = xT_d.ap().rearrange("(k p) s -> k p s", p=128)
            x_sb = []
            for k in range(KT):
                x_sb.append(xp.tile([128, SP], BF16, tag=f"x{k}", name=f"x_{k}"))
                eng = nc.sync if k % 2 == 0 else nc.gpsimd
                eng.dma_start(x_sb[k][:], xT_ap[k])
            # const tables follow on the gpsimd queue
            nc.gpsimd.dma_start(cos2_sb[:], cos2_d.ap())
            nc.gpsimd.dma_start(sinS_sb[:], sinS_d.ap())
            nc.gpsimd.dma_start(qb2_sb[:], qb2_d.ap())
            nc.gpsimd.dma_start(dqb2_sb[:], dqb2_d.ap())
            nc.gpsimd.dma_start(vb_sb[:], vb_d.ap())
            nc.gpsimd.dma_start(ob_sb[:], ob_d.ap())

            def emit_copy(ps, c0, w, m, dst, raw, bias2, det_bias2):
                """Evict one PSUM proj chunk: prefix -> dst, roped -> raw (bf16)."""
                nr1 = min(c0 + w, R0)
                if nr1 > c0:
                    b = det_bias2 if c0 < NDET else bias2
                    if b is None:
                        nc.vector.tensor_copy(dst[:, c0:nr1], ps[:, 0:nr1 - c0])
                    else:
                        nc.vector.tensor_scalar_add(dst[:, c0:nr1],
                                                    ps[:, 0:nr1 - c0],
                                                    b[:, m:m + 1])
                r0, r1 = max(c0, R0), min(c0 + w, S)
                if r1 <= r0:
                    return
                rw, o0, t0 = r1 - r0, r0 - c0, r0 - R0
                if bias2 is None:
                    nc.vector.tensor_copy(raw[:, t0:t0 + rw], ps[:, o0:o0 + rw])
                else:
                    # roped tokens are all past the det prefix -> plain q bias
                    nc.vector.tensor_scalar_add(raw[:, t0:t0 + rw],
                                                ps[:, o0:o0 + rw],
                                                bias2[:, m:m + 1])

            def emit_rope_wide(raw, dst):
                """RoPE over the whole roped range in bf16 2x-mode DVE ops."""
                qsw = rtmp.tile([128, P], BF16, tag="qsw")
                tsin = rtmp.tile([128, P], BF16, tag="tsin")
                tcos = rtmp.tile([128, P], BF16, tag="tcos")
                nc.vector.stream_shuffle(qsw[:], raw[:], mask=SWAP_MASK)
                nc.vector.tensor_mul(tsin[:], qsw[:], sinS_sb[:])
                nc.vector.tensor_mul(tcos[:], raw[:], cos2_sb[:])
                nc.vector.tensor_add(dst[:, R0:S], tcos[:], tsin[:])

            ao_tiles = {}

            def proj_units(pair, wsb_m=None):
                """q/k projections + RoPE for E-tile `pair`, as 9 callable
                units so they can be zipped into the previous pair's
                exp-paced scores window."""
                m = pair
                if wsb_m is None:
                    wsb_m = load_w(pair)
                qT_p = qkp.tile([128, SP], BF16, tag="qT", name=f"qT_{pair}")
                kT_p = qkp.tile([128, SP], BF16, tag="kT", name=f"kT_{pair}")
                units = []
                for ci, (chunks, dst, bias2, det_bias2) in enumerate((
                    (Q_CHUNKS, qT_p, qb2_sb, dqb2_sb),
                    (K_CHUNKS, kT_p, None, None),
                )):
                    raw = rtmp.tile([128, P], BF16, tag="raw",
                                    name=f"raw_{pair}_{ci}")

                    def chunk_unit(c0, w, which, dst=dst, raw=raw,
                                   bias2=bias2, det_bias2=det_bias2):
                        ps = psA.tile([128, 512], F32, tag="pj", bufs=2)
                        wsb = wsb_m[which]
                        for k in range(KT):
                            nc.tensor.matmul(
                                ps[:, :w], wsb[:, k, :],
                                x_sb[k][:, c0:c0 + w],
                                start=(k == 0), stop=(k == KT - 1))
                        emit_copy(ps, c0, w, m, dst, raw, bias2, det_bias2)

                    for c0, w, which in chunks:
                        units.append(lambda c0=c0, w=w, which=which,
                                     f=chunk_unit: f(c0, w, which))
                    units.append(lambda raw=raw, dst=dst:
                                 emit_rope_wide(raw, dst))
                attn2 = attnp.tile([128, ST, 2, SP], BF16, tag="attn",
                                   name=f"attn_{pair}")
                return qT_p, kT_p, attn2, units

            def emit_proj(pair, wsb_m=None):
                qT_p, kT_p, attn2, units = proj_units(pair, wsb_m)
                for u in units:
                    u()
                return qT_p, kT_p, attn2

            def emit_scores_kt(qT_p, kT_p, attn2, kt):
                """scores^T + exp for one k-tile."""
                kw = _sw(kt)
                for qc0, qw in SC_CHUNKS:
                    ps = psA.tile([128, 1024], F32, tag="sc", bufs=2)
                    for hp in range(2):
                        row0 = hp * 64
                        nc.tensor.matmul(
                            ps[:kw, hp * 512:hp * 512 + qw],
                            kT_p[row0:row0 + 64, kt * 128:kt * 128 + kw],
                            qT_p[row0:row0 + 64, qc0:qc0 + qw],
                            start=True, stop=True, tile_position=(row0, 0))
                    # one wide exp covering both heads (strided dst)
                    src = ps[:kw, :].rearrange("p (h q) -> p h q", h=2)
                    nc.scalar.activation(
                        attn2[:kw, kt, :, qc0:qc0 + qw],
                        src[:, :, 0:qw], AF.Exp, scale=SCALE)

            def emit_av_qt(pair, attn2, qt):
                """attn @ [v|1] + normalize for one q-tile of `pair`."""
                if pair not in ao_tiles:
                    ao_tiles[pair] = aop.tile([128, ST, 128], BF16, tag="ao",
                                              name=f"ao_{pair}")
                ao_sb = ao_tiles[pair]
                qw = _sw(qt)
                # last tile: write 112 rows (16-aligned) so the XBAR
                # transpose below reads initialized data
                qwp = 128 if qt < ST - 1 else 112
                pso = psA.tile([128, 130], F32, tag="av", bufs=2)
                for hp in range(2):
                    head = pair * 2 + hp
                    for kt in range(ST):
                        kw = _sw(kt)
                        nc.tensor.matmul(
                            pso[:qw, hp * 65:hp * 65 + 65],
                            attn2[:kw, kt, hp, qt * 128:qt * 128 + qw],
                            v_sb[:kw, kt, head * 65:(head + 1) * 65],
                            start=(kt == 0), stop=(kt == ST - 1))
                rec = nrm.tile([128, 2], F32, tag="rec")
                nc.vector.reciprocal(
                    rec[:qwp, :],
                    pso[:qwp, :].rearrange("p (h c) -> p h c", h=2)[:, :, 64])
                for hp in range(2):
                    nc.vector.tensor_scalar_mul(
                        ao_sb[:qwp, qt, hp * 64:(hp + 1) * 64],
                        pso[:qwp, hp * 65:hp * 65 + 64], rec[:qwp, hp:hp + 1])

            def emit_tr_st(pair, st):
                """One st-tile of token-major -> feature-major XBAR DMA."""
                ao_sb = ao_tiles[pair]
                swp = 128 if st < ST - 1 else 112
                nc.sync.dma_start_transpose(
                    aT_sb[pair][:, st * 128:st * 128 + swp],
                    ao_sb[0:swp, st, 0:128])

            def emit_transpose(pair):
                for st in range(ST):
                    emit_tr_st(pair, st)
                ao_tiles.pop(pair)

            def emit_oproj_st(st):
                sw = _sw(st)
                out_t = outp.tile([128, E], F32, tag="out")
                for c0, w in E_CHUNKS:
                    ps = psA.tile([128, 512], F32, tag="pj", bufs=2)
                    for et in range(KT):
                        nc.tensor.matmul(
                            ps[:sw, :w], aT_sb[et][:, st * 128:st * 128 + sw],
                            wo_sb[:, et, c0:c0 + w],
                            start=(et == 0), stop=False)
                    nc.tensor.matmul(ps[:sw, :w], ones_sb[0:1, :sw],
                                     ob_sb[0:1, c0:c0 + w], start=False, stop=True)
                    nc.scalar.copy(out_t[:sw, c0:c0 + w], ps[:sw, :w])
                nc.sync.dma_start(out_d.ap()[st * 128:st * 128 + sw, :],
                                  out_t[:sw, :])

            # pairs 0+1 proj+scores first so ScalarE exp covers the v phase
            q0, k0, a0 = emit_proj(0, w0)
            for kt in range(ST):
                emit_scores_kt(q0, k0, a0, kt)
            q1, k1, a1 = emit_proj(1)
            for kt in range(ST):
                emit_scores_kt(q1, k1, a1, kt)
            wv_sb = wvo.tile([128, KT, E], BF16, tag="wvo", name="wv")
            nc.gpsimd.dma_start(wv_sb[:], wv_d.ap())
            for st in range(ST):  # v token-major
                sw = _sw(st)
                for c0, w in E_CHUNKS:
                    ps = psA.tile([128, 512], F32, tag="pj", bufs=2)
                    for k in range(KT):
                        nc.tensor.matmul(
                            ps[:sw, :w], x_sb[k][:, st * 128:st * 128 + sw],
                            wv_sb[:, k, c0:c0 + w],
                            start=(k == 0), stop=False)
                    nc.tensor.matmul(ps[:sw, :w], ones_sb[0:1, :sw],
                                     vb_sb[0:1, c0:c0 + w], start=False, stop=True)
                    h0 = c0 // 64
                    nhead = w // 64
                    dst = v4[0:sw, st, h0:h0 + nhead, 0:64]
                    src = ps[:sw, :w].rearrange("p (h d) -> p h d", d=64)
                    nc.vector.tensor_copy(dst, src)
            # wo reuses wv's buffer; the DMA waits for the last v matmul read
            wo_sb = wvo.tile([128, KT, E], BF16, tag="wvo", name="wo")
            nc.gpsimd.dma_start(wo_sb[:], wo_d.ap())

            # steady state: av(p-2) fully drains before scores(p) writes into
            # its attn buffer (2-pair lookahead, attnp bufs=2)
            attn_q = [(0, a0), (1, a1)]
            for pair in range(2, H // 2):
                pv, at = attn_q.pop(0)
                for qt in range(ST):
                    emit_av_qt(pv, at, qt)
                emit_transpose(pv)
                qT_p, kT_p, attn2 = emit_proj(pair)
                for kt in range(ST):
                    emit_scores_kt(qT_p, kT_p, attn2, kt)
                attn_q.append((pair, attn2))
            # drain pairs 6+7 interleaved with o_proj per st-tile: the o_proj
            # chain for st starts as soon as the last transposes of st land
            (pv6, at6), (pv7, at7) = attn_q
            for qt in range(ST):
                emit_av_qt(pv6, at6, qt)
                emit_tr_st(pv6, qt)
                emit_av_qt(pv7, at7, qt)
                emit_tr_st(pv7, qt)
                emit_oproj_st(qt)
            ao_tiles.pop(pv6)
            ao_tiles.pop(pv7)

    nc.compile()
    return nc


def _prep_inputs(inputs):
    bf = ml_dtypes.bfloat16
    hs = np.asarray(inputs["hidden_states"], np.float32)
    det = np.asarray(inputs["det_tokens"], np.float32)
    cos = np.asarray(inputs["cos"], np.float32)
    sin = np.asarray(inputs["sin"], np.float32)
    q_w = np.asarray(inputs["q_w"], np.float32)
    q_b = np.asarray(inputs["q_b"], np.float32)
    k_w = np.asarray(inputs["k_w"], np.float32)
    v_w = np.asarray(inputs["v_w"], np.float32)
    v_b = np.asarray(inputs["v_b"], np.float32)
    dq_w = np.asarray(inputs["dq_w"], np.float32)
    dq_b = np.asarray(inputs["dq_b"], np.float32)
    o_w = np.asarray(inputs["o_w"], np.float32)
    o_b = np.asarray(inputs["o_b"], np.float32)

    x = np.concatenate([det, hs], axis=1)            # [B, S, E]
    xT = np.zeros((B, E, SP), bf)
    xT[:, :, :S] = x.transpose(0, 2, 1).astype(bf)

    def qkv_pack(w):  # w: [E_out(perm), E_in] -> w.T [k,p][m,e] -> [m,p,k,e]
        t = np.ascontiguousarray(w.T).astype(bf)      # [E_in, E_out]
        return np.ascontiguousarray(
            t.reshape(KT, 128, KT, 128).transpose(2, 1, 0, 3))

    def pk_pack(w):  # w.T [E_in, E_out] -> [p, k, e]
        t = np.ascontiguousarray(w.T).astype(bf)
        return np.ascontiguousarray(t.reshape(KT, 128, E).transpose(1, 0, 2))

    wq = qkv_pack(q_w[PERM_E, :])
    wdq = qkv_pack(dq_w[PERM_E, :])
    wk = qkv_pack(k_w[PERM_E, :])
    wv = pk_pack(v_w)
    wo = pk_pack(o_w)

    cosP = cos.T[PERM64]                              # [64, P]
    sinP = sin.T[PERM64]
    sign = np.where((np.arange(64) % 32) < 16, -1.0, 1.0).astype(np.float32)
    sinSP = sinP * sign[:, None]
    cos2 = np.ascontiguousarray(np.vstack([cosP, cosP])).astype(bf)
    sinS = np.ascontiguousarray(np.vstack([sinSP, sinSP])).astype(bf)

    def b2(v):
        return np.ascontiguousarray(v.reshape(KT, 128).T)

    qb_p = q_b[PERM_E]
    qb_sw = qb_p.reshape(-1, 2, 16)[:, ::-1, :].reshape(E)
    dqb_p = dq_b[PERM_E]

    common = {
        "wq": wq, "wdq": wdq, "wk": wk, "wv": wv, "wo": wo,
        "cos2": cos2, "sinS": sinS,
        "qb2": b2(qb_p), "qb2s": b2(qb_sw), "dqb2": b2(dqb_p),
        "vb": v_b.astype(bf).reshape(1, E), "ob": o_b.astype(bf).reshape(1, E),
    }
    return [dict(common, xT=np.ascontiguousarray(xT[b])) for b in range(B)]


def kernel(**inputs) -> np.ndarray:
    if "nc" not in _CACHE:
        _CACHE["nc"] = _build()
    nc = _CACHE["nc"]
    in_maps = _prep_inputs(inputs)
    res = run_bass_kernel_spmd(nc, in_maps, list(range(B)))
    return np.stack([res.results[b]["out"] for b in range(B)]).astype(np.float32)


# revision 34
# speedup vs baseline: 1.0906x; 1.0906x over previous
"""DINOv3 ViT attention (det tokens + hidden, partial RoPE) on 8 Trainium2 cores.

Strategy: data-parallel over batch (B=8 -> 1 batch element per core).
Per core, everything is bf16 on the matmul paths with fp32 accumulation:
  phase 1: q^T,k^T (feature-major, weights stationary) + RoPE fused from PSUM;
           v token-major (x^T stationary) with per-head 65-column layout
           (64 dims + a ones column used to compute softmax denominators).
  phase 2: per head: scores^T = k^T.T @ q^T (row-packed 2 heads in PE),
           exp on ScalarE (one wide ACTIVATE covering both heads via a
           strided dst AP; scale=1/8 folded),
           out_h = exp_sT.T @ [v_h | 1] accumulated over k tiles, then
           normalize by the ones-column sum (reciprocal + tensor_scalar).
  phase 3 (fused into the same context): PE-transpose attn_out to
           feature-major, o_proj (mapping B) + bias, DMA out.

RoPE trick: rotate_half needs a cross-partition swap of +-32 which DVE's
stream_shuffle cannot do (it permutes lanes within 32-partition blocks only).
We permute the head-dim order of q/k on the host (rows of q_w/dq_w/k_w and the
cos/sin tables) as [0:16, 32:48, 16:32, 48:64] per head, which puts every
rotate pair +-16 apart inside one 32-lane block. Scores are invariant to a
consistent d-permutation, so nothing downstream changes.

Weights are repacked on the host so every per-pair slice is a contiguous
DRAM region (2KB/partition descriptors instead of 256B strided reads).
"""
import numpy as np
import ml_dtypes
from contextlib import ExitStack

import concourse.mybir as mybir
import concourse.tile as tile
from concourse import bacc
from concourse.bass_utils import run_bass_kernel_spmd
from concourse.masks import make_identity

BF16 = mybir.dt.bfloat16
F32 = mybir.dt.float32
AF = mybir.ActivationFunctionType
OP = mybir.AluOpType

B = 8
NDET = 100
NHID = 1029
S = NDET + NHID            # 1129
SP = S + 1                 # 1130, padded (pad col never read)
E = 1024
H = 16
HD = 64
P = 1024
R0 = S - P                 # 105: first roped token
KT = E // 128              # 8
ST = (S + 127) // 128      # 9
SCALE = HD ** -0.5

Q_CHUNKS = [(0, 100, "dq"), (100, 512, "q"), (612, 512, "q"), (1124, 6, "q")]
K_CHUNKS = [(0, 512, "k"), (512, 512, "k"), (1024, 106, "k")]
SC_CHUNKS = [(0, 512), (512, 512), (1024, 105)]
E_CHUNKS = [(0, 512), (512, 512)]

# rotate_half pairs land +-16 apart within 32-lane blocks (see module docstring)
PERM64 = np.concatenate([np.arange(0, 16), np.arange(32, 48),
                         np.arange(16, 32), np.arange(48, 64)])
PERM_E = np.concatenate([h * 64 + PERM64 for h in range(H)])
SWAP_MASK = [(l + 16) % 32 for l in range(32)]

_CACHE = {}


def _sw(t, n=ST, full=128, last=S - 8 * 128):
    return full if t < n - 1 else last


def _build():
    nc = bacc.Bacc("TRN2", target_bir_lowering=False, debug=False, num_devices=B)
    dp = nc.declare_dram_parameter
    xT_d = dp("xT", [E, SP], BF16, False)
    # qkv weights pre-sliced per output E-tile: [m, p, k, e] contiguous
    wq_d = dp("wq", [KT, 128, KT, 128], BF16, False)
    wdq_d = dp("wdq", [KT, 128, KT, 128], BF16, False)
    wk_d = dp("wk", [KT, 128, KT, 128], BF16, False)
    # v/o weights pre-transposed to [p, k, e] contiguous
    wv_d = dp("wv", [128, KT, E], BF16, False)
    wo_d = dp("wo", [128, KT, E], BF16, False)
    cos2_d = dp("cos2", [128, P], BF16, False)
    sinS_d = dp("sinS", [128, P], BF16, False)
    qb2_d = dp("qb2", [128, KT], F32, False)
    qb2s_d = dp("qb2s", [128, KT], F32, False)
    dqb2_d = dp("dqb2", [128, KT], F32, False)
    vb_d = dp("vb", [1, E], BF16, False)
    ob_d = dp("ob", [1, E], BF16, False)
    out_d = dp("out", [S, E], F32, True)

    with tile.TileContext(nc) as tc, ExitStack() as octx:
        const = octx.enter_context(tc.tile_pool(name="const", bufs=1))
        qkv = octx.enter_context(tc.tile_pool(name="qkv", bufs=1))

        cos2_sb = const.tile([128, P], BF16, tag="cos2")
        sinS_sb = const.tile([128, P], BF16, tag="sinS")
        qb2_sb = const.tile([128, KT], F32, tag="qb2")
        qb2s_sb = const.tile([128, KT], F32, tag="qb2s")
        dqb2_sb = const.tile([128, KT], F32, tag="dqb2")
        vb_sb = const.tile([1, E], BF16, tag="vb")
        ob_sb = const.tile([1, E], BF16, tag="ob")
        ones_sb = const.tile([1, 128], BF16, tag="ones")
        nc.gpsimd.memset(ones_sb[:], 1.0)

        v_sb = qkv.tile([128, ST, H * 65], BF16, tag="v")
        # padded to 9*128 cols: the last 112-row XBAR transpose spills 7 cols
        aT_sb = [qkv.tile([128, ST * 128], BF16, tag=f"aT{et}", name=f"aT_{et}")
                 for et in range(KT)]  # feature-major attn out, per E-tile

        # ones column of v_aug (col 64 of each per-head 65-block)
        v4 = v_sb[:, :, :].rearrange("p s (h d) -> p s h d", d=65)
        nc.gpsimd.memset(v4[:, :, :, 64:65], 1.0)

        wapc = {"q": wq_d, "dq": wdq_d, "k": wk_d}

        # ------- phases 1+2+3 in one context -------
        with ExitStack() as ctx:
            wsl = ctx.enter_context(tc.tile_pool(name="wsl", bufs=2))
            xp = ctx.enter_context(tc.tile_pool(name="xp", bufs=1))
            qkp = ctx.enter_context(tc.tile_pool(name="qkp", bufs=3))
            psA = ctx.enter_context(tc.tile_pool(name="psA", bufs=1, space="PSUM"))
            rtmp = ctx.enter_context(tc.tile_pool(name="rtmp", bufs=1))
            attnp = ctx.enter_context(tc.tile_pool(name="attn", bufs=2))
            aop = ctx.enter_context(tc.tile_pool(name="aop", bufs=2))
            nrm = ctx.enter_context(tc.tile_pool(name="nrm", bufs=6))
            wvo = ctx.enter_context(tc.tile_pool(name="wvo", bufs=1))
            outp = ctx.enter_context(tc.tile_pool(name="outp", bufs=2))

            def load_w(pair, split=False):
                """DMA the three qkv weight slices for one E-tile."""
                wsb_m = {}
                for i, which in enumerate(("q", "dq", "k")):
                    wsb_m[which] = wsl.tile([128, KT, 128], BF16, tag="w" + which,
                                            name=f"w_{which}_{pair}")
                    eng = nc.gpsimd if (split and i > 0) else nc.sync
                    eng.dma_start(wsb_m[which][:], wapc[which].ap()[pair])
                return wsb_m

            # pair-0 weights lead both queues so the PE can start ASAP;
            # x tiles alternate across the two queues right behind them
            w0 = load_w(0, split=True)
            xT_ap = xT_d.ap().rearrange("(k p) s -> k p s", p=128)
            x_sb = []
            for k in range(KT):
                x_sb.append(xp.tile([128, SP], BF16, tag=f"x{k}", name=f"x_{k}"))
                eng = nc.sync if k % 2 == 0 else nc.gpsimd
                eng.dma_start(x_sb[k][:], xT_ap[k])
            # const tables follow on the gpsimd queue
            nc.gpsimd.dma_start(cos2_sb[:], cos2_d.ap())
            nc.gpsimd.dma_start(sinS_sb[:], sinS_d.ap())
            nc.gpsimd.dma_start(qb2_sb[:], qb2_d.ap())
            nc.gpsimd.dma_start(dqb2_sb[:], dqb2_d.ap())
            nc.gpsimd.dma_start(vb_sb[:], vb_d.ap())
            nc.gpsimd.dma_start(ob_sb[:], ob_d.ap())

            def emit_copy(ps, c0, w, m, dst, raw, bias2, det_bias2):
                """Evict one PSUM proj chunk: prefix -> dst, roped -> raw (bf16)."""
                nr1 = min(c0 + w, R0)
                if nr1 > c0:
                    b = det_bias2 if c0 < NDET else bias2
                    if b is None:
                        nc.vector.tensor_copy(dst[:, c0:nr1], ps[:, 0:nr1 - c0])
                    else:
                        nc.vector.tensor_scalar_add(dst[:, c0:nr1],
                                                    ps[:, 0:nr1 - c0],
                                                    b[:, m:m + 1])
                r0, r1 = max(c0, R0), min(c0 + w, S)
                if r1 <= r0:
                    return
                rw, o0, t0 = r1 - r0, r0 - c0, r0 - R0
                if bias2 is None:
                    nc.vector.tensor_copy(raw[:, t0:t0 + rw], ps[:, o0:o0 + rw])
                else:
                    # roped tokens are all past the det prefix -> plain q bias
                    nc.vector.tensor_scalar_add(raw[:, t0:t0 + rw],
                                                ps[:, o0:o0 + rw],
                                                bias2[:, m:m + 1])

            def emit_rope_wide(raw, dst):
                """RoPE over the whole roped range in bf16 2x-mode DVE ops."""
                qsw = rtmp.tile([128, P], BF16, tag="qsw")
                tsin = rtmp.tile([128, P], BF16, tag="tsin")
                tcos = rtmp.tile([128, P], BF16, tag="tcos")
                nc.vector.stream_shuffle(qsw[:], raw[:], mask=SWAP_MASK)
                nc.vector.tensor_mul(tsin[:], qsw[:], sinS_sb[:])
                nc.vector.tensor_mul(tcos[:], raw[:], cos2_sb[:])
                nc.vector.tensor_add(dst[:, R0:S], tcos[:], tsin[:])

            ao_tiles = {}

            def proj_units(pair, wsb_m=None):
                """q/k projections + RoPE for E-tile `pair`, as 9 callable
                units so they can be zipped into the previous pair's
                exp-paced scores window."""
                m = pair
                if wsb_m is None:
                    wsb_m = load_w(pair)
                qT_p = qkp.tile([128, SP], BF16, tag="qT", name=f"qT_{pair}")
                kT_p = qkp.tile([128, SP], BF16, tag="kT", name=f"kT_{pair}")
                units = []
                for ci, (chunks, dst, bias2, det_bias2) in enumerate((
                    (Q_CHUNKS, qT_p, qb2_sb, dqb2_sb),
                    (K_CHUNKS, kT_p, None, None),
                )):
                    raw = rtmp.tile([128, P], BF16, tag="raw",
                                    name=f"raw_{pair}_{ci}")

                    def chunk_unit(c0, w, which, dst=dst, raw=raw,
                                   bias2=bias2, det_bias2=det_bias2):
                        ps = psA.tile([128, 512], F32, tag="pa", bufs=2)
                        wsb = wsb_m[which]
                        for k in range(KT):
                            nc.tensor.matmul(
                                ps[:, :w], wsb[:, k, :],
                                x_sb[k][:, c0:c0 + w],
                                start=(k == 0), stop=(k == KT - 1))
                        emit_copy(ps, c0, w, m, dst, raw, bias2, det_bias2)

                    for c0, w, which in chunks:
                        units.append(lambda c0=c0, w=w, which=which,
                                     f=chunk_unit: f(c0, w, which))
                    units.append(lambda raw=raw, dst=dst:
                                 emit_rope_wide(raw, dst))
                attn2 = attnp.tile([128, ST, 2, SP], BF16, tag="attn",
                                   name=f"attn_{pair}")
                return qT_p, kT_p, attn2, units

            def emit_proj(pair, wsb_m=None):
                qT_p, kT_p, attn2, units = proj_units(pair, wsb_m)
                for u in units:
                    u()
                return qT_p, kT_p, attn2

            def emit_scores_kt(qT_p, kT_p, attn2, kt):
                """scores^T + exp for one k-tile."""
                kw = _sw(kt)
                for qc0, qw in SC_CHUNKS:
                    ps = psA.tile([128, 1024], F32, tag="sc", bufs=3)
                    for hp in range(2):
                        row0 = hp * 64
                        nc.tensor.matmul(
                            ps[:kw, hp * 512:hp * 512 + qw],
                            kT_p[row0:row0 + 64, kt * 128:kt * 128 + kw],
                            qT_p[row0:row0 + 64, qc0:qc0 + qw],
                            start=True, stop=True, tile_position=(row0, 0))
                    # one wide exp covering both heads (strided dst)
                    src = ps[:kw, :].rearrange("p (h q) -> p h q", h=2)
                    nc.scalar.activation(
                        attn2[:kw, kt, :, qc0:qc0 + qw],
                        src[:, :, 0:qw], AF.Exp, scale=SCALE)

            def emit_av_qt(pair, attn2, qt):
                """attn @ [v|1] + normalize for one q-tile of `pair`."""
                if pair not in ao_tiles:
                    ao_tiles[pair] = aop.tile([128, ST, 128], BF16, tag="ao",
                                              name=f"ao_{pair}")
                ao_sb = ao_tiles[pair]
                qw = _sw(qt)
                # last tile: write 112 rows (16-aligned) so the XBAR
                # transpose below reads initialized data
                qwp = 128 if qt < ST - 1 else 112
                pso = psA.tile([128, 130], F32, tag="pa", bufs=2)
                for hp in range(2):
                    head = pair * 2 + hp
                    for kt in range(ST):
                        kw = _sw(kt)
                        nc.tensor.matmul(
                            pso[:qw, hp * 65:hp * 65 + 65],
                            attn2[:kw, kt, hp, qt * 128:qt * 128 + qw],
                            v_sb[:kw, kt, head * 65:(head + 1) * 65],
                            start=(kt == 0), stop=(kt == ST - 1))
                rec = nrm.tile([128, 2], F32, tag="rec")
                nc.vector.reciprocal(
                    rec[:qwp, :],
                    pso[:qwp, :].rearrange("p (h c) -> p h c", h=2)[:, :, 64])
                for hp in range(2):
                    nc.vector.tensor_scalar_mul(
                        ao_sb[:qwp, qt, hp * 64:(hp + 1) * 64],
                        pso[:qwp, hp * 65:hp * 65 + 64], rec[:qwp, hp:hp + 1])

            def emit_tr_st(pair, st):
                """One st-tile of token-major -> feature-major XBAR DMA."""
                ao_sb = ao_tiles[pair]
                swp = 128 if st < ST - 1 else 112
                nc.sync.dma_start_transpose(
                    aT_sb[pair][:, st * 128:st * 128 + swp],
                    ao_sb[0:swp, st, 0:128])

            def emit_transpose(pair):
                for st in range(ST):
                    emit_tr_st(pair, st)
                ao_tiles.pop(pair)

            def emit_oproj_st(st):
                sw = _sw(st)
                out_t = outp.tile([128, E], F32, tag="out")
                for c0, w in E_CHUNKS:
                    ps = psA.tile([128, 512], F32, tag="pa", bufs=2)
                    for et in range(KT):
                        nc.tensor.matmul(
                            ps[:sw, :w], aT_sb[et][:, st * 128:st * 128 + sw],
                            wo_sb[:, et, c0:c0 + w],
                            start=(et == 0), stop=False)
                    nc.tensor.matmul(ps[:sw, :w], ones_sb[0:1, :sw],
                                     ob_sb[0:1, c0:c0 + w], start=False, stop=True)
                    nc.scalar.copy(out_t[:sw, c0:c0 + w], ps[:sw, :w])
                nc.sync.dma_start(out_d.ap()[st * 128:st * 128 + sw, :],
                                  out_t[:sw, :])

            # pairs 0+1 proj+scores first so ScalarE exp covers the v phase
            q0, k0, a0 = emit_proj(0, w0)
            for kt in range(ST):
                emit_scores_kt(q0, k0, a0, kt)
            q1, k1, a1 = emit_proj(1)
            for kt in range(ST):
                emit_scores_kt(q1, k1, a1, kt)
            wv_sb = wvo.tile([128, KT, E], BF16, tag="wvo", name="wv")
            nc.gpsimd.dma_start(wv_sb[:], wv_d.ap())
            for st in range(ST):  # v token-major
                sw = _sw(st)
                for c0, w in E_CHUNKS:
                    ps = psA.tile([128, 512], F32, tag="pa", bufs=2)
                    for k in range(KT):
                        nc.tensor.matmul(
                            ps[:sw, :w], x_sb[k][:, st * 128:st * 128 + sw],
                            wv_sb[:, k, c0:c0 + w],
                            start=(k == 0), stop=False)
                    nc.tensor.matmul(ps[:sw, :w], ones_sb[0:1, :sw],
                                     vb_sb[0:1, c0:c0 + w], start=False, stop=True)
                    h0 = c0 // 64
                    nhead = w // 64
                    dst = v4[0:sw, st, h0:h0 + nhead, 0:64]
                    src = ps[:sw, :w].rearrange("p (h d) -> p h d", d=64)
                    nc.vector.tensor_copy(dst, src)
            # wo reuses wv's buffer; the DMA waits for the last v matmul read
            wo_sb = wvo.tile([128, KT, E], BF16, tag="wvo", name="wo")
            nc.gpsimd.dma_start(wo_sb[:], wo_d.ap())

            # steady state: av(p-2) fully drains before scores(p) writes into
            # its attn buffer (2-pair lookahead, attnp bufs=2)
            attn_q = [(0, a0), (1, a1)]
            for pair in range(2, H // 2):
                pv, at = attn_q.pop(0)
                for qt in range(ST):
                    emit_av_qt(pv, at, qt)
                emit_transpose(pv)
                qT_p, kT_p, attn2 = emit_proj(pair)
                for kt in range(ST):
                    emit_scores_kt(qT_p, kT_p, attn2, kt)
                attn_q.append((pair, attn2))
            for pv, at in attn_q:
                for qt in range(ST):
                    emit_av_qt(pv, at, qt)
                emit_transpose(pv)
            for st in range(ST):
                emit_oproj_st(st)

    nc.compile()
    return nc


def _prep_inputs(inputs):
    bf = ml_dtypes.bfloat16
    hs = np.asarray(inputs["hidden_states"], np.float32)
    det = np.asarray(inputs["det_tokens"], np.float32)
    cos = np.asarray(inputs["cos"], np.float32)
    sin = np.asarray(inputs["sin"], np.float32)
    q_w = np.asarray(inputs["q_w"], np.float32)
    q_b = np.asarray(inputs["q_b"], np.float32)
    k_w = np.asarray(inputs["k_w"], np.float32)
    v_w = np.asarray(inputs["v_w"], np.float32)
    v_b = np.asarray(inputs["v_b"], np.float32)
    dq_w = np.asarray(inputs["dq_w"], np.float32)
    dq_b = np.asarray(inputs["dq_b"], np.float32)
    o_w = np.asarray(inputs["o_w"], np.float32)
    o_b = np.asarray(inputs["o_b"], np.float32)

    x = np.concatenate([det, hs], axis=1)            # [B, S, E]
    xT = np.zeros((B, E, SP), bf)
    xT[:, :, :S] = x.transpose(0, 2, 1).astype(bf)

    def qkv_pack(w):  # w: [E_out(perm), E_in] -> w.T [k,p][m,e] -> [m,p,k,e]
        t = np.ascontiguousarray(w.T).astype(bf)      # [E_in, E_out]
        return np.ascontiguousarray(
            t.reshape(KT, 128, KT, 128).transpose(2, 1, 0, 3))

    def pk_pack(w):  # w.T [E_in, E_out] -> [p, k, e]
        t = np.ascontiguousarray(w.T).astype(bf)
        return np.ascontiguousarray(t.reshape(KT, 128, E).transpose(1, 0, 2))

    wq = qkv_pack(q_w[PERM_E, :])
    wdq = qkv_pack(dq_w[PERM_E, :])
    wk = qkv_pack(k_w[PERM_E, :])
    wv = pk_pack(v_w)
    wo = pk_pack(o_w)

    cosP = cos.T[PERM64]                              # [64, P]
    sinP = sin.T[PERM64]
    sign = np.where((np.arange(64) % 32) < 16, -1.0, 1.0).astype(np.float32)
    sinSP = sinP * sign[:, None]
    cos2 = np.ascontiguousarray(np.vstack([cosP, cosP])).astype(bf)
    sinS = np.ascontiguousarray(np.vstack([sinSP, sinSP])).astype(bf)

    def b2(v):
        return np.ascontiguousarray(v.reshape(KT, 128).T)

    qb_p = q_b[PERM_E]
    qb_sw = qb_p.reshape(-1, 2, 16)[:, ::-1, :].reshape(E)
    dqb_p = dq_b[PERM_E]

    common = {
        "wq": wq, "wdq": wdq, "wk": wk, "wv": wv, "wo": wo,
        "cos2": cos2, "sinS": sinS,
        "qb2": b2(qb_p), "qb2s": b2(qb_sw), "dqb2": b2(dqb_p),
        "vb": v_b.astype(bf).reshape(1, E), "ob": o_b.astype(bf).reshape(1, E),
    }
    return [dict(common, xT=np.ascontiguousarray(xT[b])) for b in range(B)]


def kernel(**inputs) -> np.ndarray:
    if "nc" not in _CACHE:
        _CACHE["nc"] = _build()
    nc = _CACHE["nc"]
    in_maps = _prep_inputs(inputs)
    res = run_bass_kernel_spmd(nc, in_maps, list(range(B)))
    return np.stack([res.results[b]["out"] for b in range(B)]).astype(np.float32)
